# revision 20
# baseline (speedup 1.0000x reference)
"""Grouped SwiGLU MoE expert FFN on 8 Trainium2 NeuronCores.

Problem: out[t] = w2[e(t)] @ (silu(w1[e(t)] x[t]) * (w3[e(t)] x[t])),
T=4096 tokens sorted by expert, E=8 experts, H=1024, I=2816, fp32.

Strategy — fp8 hi/lo DoubleRow matmuls + fractional tensor parallelism:

  * Precision: every matmul operand (w1/w3/w2, x, and the intermediate h)
    is split into fp8e4m3 hi + lo parts (lo = exact residual of hi), and
    each logical matmul computes the three significant cross terms
    hi*hi + lo*hi + hi*lo with fp32 PSUM accumulation.  Measured end-to-end
    rel-L2 vs the fp32 reference: ~2.8e-3.  PE runs these as DoubleRow fp8
    matmuls (two independent 128-deep lanes per instruction at 0.5
    cycles/output-column), i.e. 1.33x the fp16 MAC rate.
  * Distribution: the work grid (token-block x I-column-block) of each
    expert is covered exactly by cells of (512|384|128 tokens) x 1408
    I-columns (half of I).  Each of the 8 cores gets one cell of each size,
    so compute is perfectly balanced (1024 tokens x half-I per core) and
    each core streams only 1.5 experts-worth of weights (vs 2.0 for whole-
    expert assignment).  SwiGLU is elementwise per I-column, so the column
    split is exact; each token's output is the sum of its two cells'
    partials (summed on the host).
  * Weight/activation scales are powers of two (lossless): w1*32, w3*16,
    w2*32; silu is evaluated with ACT scale 1/32; the stored h = 16*h_true
    keeps fp8 h in the normal range; outputs are descaled by 2^-9 on chip.

Self-contained: only needs numpy + the concourse/axon runtime.
"""

import numpy as np
import ml_dtypes

import jax
import concourse.tile as tile
from concourse import bacc, mybir

E, H, II = 8, 1024, 2816
NCORES = 8
BLK = 128
KT = H // BLK          # 8 contraction blocks (stage 1)
IHALF = II // 2        # 1408 columns per cell
IT = IHALF // BLK      # 11 intermediate blocks per cell
MT = H // BLK          # 8 output blocks
CHUNKS = (512, 384, 128)   # tokens per chunk (one cell each per core)

F32 = mybir.dt.float32
FP16 = mybir.dt.float16
FP8 = mybir.dt.float8e4
E4 = ml_dtypes.float8_e4m3
DR = mybir.MatmulPerfMode.DoubleRow

S1 = 32.0   # w1 scale
S3 = 16.0   # w3 scale (also provides the 16x h storage scale)
S2 = 32.0   # w2 scale
OUT_DESCALE = 1.0 / (S2 * 16.0)   # 2^-9


# ----------------------------------------------------------------------------
# Planning: cover each expert's (token-block x {L,R}) grid with cells of
# 4/3/1 token-blocks, 8 cells of each size globally (one per core each).
# ----------------------------------------------------------------------------

def _plan_cells(group_sizes):
    """Return {size_blocks: [(expert, tok_start_blk, side), ...*8]} or None.

    Each expert side (L/R halves of I) of g blocks is decomposed into parts
    from {4,3,1}; globally exactly 8 parts of each size must be used.
    """
    g = [int(v) // BLK for v in group_sizes]
    if sum(v * BLK for v in g) != 4096 or any(int(v) % BLK for v in group_sizes):
        return None

    def decomps(n):
        res = []
        for n4 in range(n // 4 + 1):
            for n3 in range((n - 4 * n4) // 3 + 1):
                res.append((n4, n3, n - 4 * n4 - 3 * n3))
        return res

    # DP over the 16 sides; state = (#4s used, #3s used)
    sides = [d for e in range(E) for d in (("L", e), ("R", e))]
    from functools import lru_cache

    opts = {e: decomps(g[e]) for e in range(E)}

    @lru_cache(maxsize=None)
    def solve(idx, used4, used3):
        if used4 > 8 or used3 > 8:
            return None
        if idx == len(sides):
            return () if (used4 == 8 and used3 == 8) else None
        _, e = sides[idx]
        for d in opts[e]:
            rest = solve(idx + 1, used4 + d[0], used3 + d[1])
            if rest is not None:
                return (d,) + rest
        return None

    sol = solve(0, 0, 0)
    if sol is None:
        return None
    cells = {4: [], 3: [], 1: []}
    offs = np.concatenate([[0], np.cumsum(g)])
    for (side, e), (n4, n3, n1) in zip(sides, sol):
        t = 0
        for size, cnt in ((4, n4), (3, n3), (1, n1)):
            for _ in range(cnt):
                cells[size].append((e, int(offs[e]) + t, side))
                t += size
        assert t == g[e]
    if any(len(v) != 8 for v in cells.values()):
        return None
    return cells


# ----------------------------------------------------------------------------
# Device program (identical on all cores)
# ----------------------------------------------------------------------------

def _build_program():
    nc = bacc.Bacc()

    xhs, xls, w13hs, w2s, outs = [], [], [], [], []
    for j, nr in enumerate(CHUNKS):
        xhs.append(nc.declare_dram_parameter(f"xh_{j}", [128, KT, nr], FP8, isOutput=False))
        xls.append(nc.declare_dram_parameter(f"xl_{j}", [128, KT, nr], FP8, isOutput=False))
        w13hs.append(nc.declare_dram_parameter(f"w13_{j}", [IT, 128, 4 * KT, BLK], FP8, isOutput=False))
        w2s.append(nc.declare_dram_parameter(f"w2_{j}", [MT // 2, 128, 2, 2 * IT + 1, BLK], FP8, isOutput=False))
        if nr >= 4 * BLK:
            outs.append(nc.declare_dram_parameter(f"out_{j}", [MT, 128, nr], FP16, isOutput=True))
        else:  # pack m-tile pairs so DMA descriptors stay >= 512B
            outs.append(nc.declare_dram_parameter(f"out_{j}", [MT // 2, 128, 2 * nr], FP16, isOutput=True))

    with tile.TileContext(nc) as tc:
        with tc.tile_pool(name="xp", bufs=1) as xp, \
             tc.tile_pool(name="w13", bufs=5) as w13p, \
             tc.tile_pool(name="w2", bufs=3 * MT // 2 + 1) as w2p, \
             tc.tile_pool(name="h", bufs=1) as hp, \
             tc.tile_pool(name="su", bufs=3) as sup, \
             tc.tile_pool(name="ht", bufs=3) as htp, \
             tc.tile_pool(name="oc", bufs=12) as ocp, \
             tc.tile_pool(name="up", bufs=2, space="PSUM") as up, \
             tc.tile_pool(name="vp", bufs=2, space="PSUM") as vp, \
             tc.tile_pool(name="op", bufs=4, space="PSUM") as op:

            # PE clock warm-up: sized to end just after the first x/w13 DMAs
            # land so the array hits full p-state with no idle gap.
            warm = sup.tile([1, 64], FP8, tag="warm")
            nc.vector.memset(warm[:], 0.0)
            wps = op.tile([64, 64], F32, tag="o", name="warmps")
            for _ in range(68):
                nc.tensor.matmul(wps[:], warm[:], warm[:], start=True, stop=True)

            # x tiles: hi and lo as separate tiles/DMAs (smaller critical head)
            xht, xlt, hts = [], [], []
            for j, nr in enumerate(CHUNKS):
                xht.append(xp.tile([128, KT, nr], FP8, name=f"xh{j}", tag=f"xh{j}"))
                xlt.append(xp.tile([128, KT, nr], FP8, name=f"xl{j}", tag=f"xl{j}"))
                hts.append(hp.tile([128, 2 * IT + 1, nr], FP8, name=f"h{j}", tag=f"h{j}"))

            w13ts = {}

            def dma_w13(j, i):
                wt = w13p.tile([128, 4 * KT, BLK], FP8, tag="w13")
                nc.sync.dma_start(out=wt[:], in_=w13hs[j][i])
                w13ts[j, i] = wt

            # DMA issue order: split the first x/w13 tiles so the first
            # matmuls can start ~4.3us in; then stream the rest, slotting
            # x1/x2 late enough not to delay the w13 stream.
            nc.sync.dma_start(out=xht[0][:, 0: KT // 2, :], in_=xhs[0][:, 0: KT // 2, :])
            w00 = w13p.tile([128, 4 * KT, BLK], FP8, tag="w13")
            nc.sync.dma_start(out=w00[:, 0: 2 * KT, :], in_=w13hs[0][0, :, 0: 2 * KT, :])
            nc.sync.dma_start(out=xht[0][:, KT // 2:, :], in_=xhs[0][:, KT // 2:, :])
            nc.sync.dma_start(out=w00[:, 2 * KT:, :], in_=w13hs[0][0, :, 2 * KT:, :])
            w13ts[0, 0] = w00
            dma_w13(0, 1)
            nc.sync.dma_start(out=xlt[0][:], in_=xls[0][:])
            for i in range(2, 4):
                dma_w13(0, i)
            nc.sync.dma_start(out=xht[1][:], in_=xhs[1][:])
            nc.sync.dma_start(out=xlt[1][:], in_=xls[1][:])
            for i in range(4, 7):
                dma_w13(0, i)
            nc.sync.dma_start(out=xht[2][:], in_=xhs[2][:])
            nc.sync.dma_start(out=xlt[2][:], in_=xls[2][:])
            for i in range(7, IT):
                dma_w13(0, i)
            for j in (1, 2):
                for i in range(IT):
                    dma_w13(j, i)

            def stage1(j):
                nr = CHUNKS[j]
                xh, xl = xht[j], xlt[j]
                psums = {}

                def hh_lh(i):
                    wt = w13ts[j, i]
                    u = up.tile([128, nr], F32, tag="u")
                    v = vp.tile([128, nr], F32, tag="v")
                    psums[i] = [u, v, 0, 0]
                    st = psums[i]

                    def mm(pi, ps, stat, mv):
                        st[2 + pi] += 1
                        nc.tensor.matmul(ps[:], stat, mv,
                                         start=(st[2 + pi] == 1), stop=False,
                                         perf_mode=DR)

                    for half in range(2):        # hh: w_hi x x_hi (u,v per half)
                        for t in (2 * half, 2 * half + 1):
                            mm(0, u, wt[:, 2 * t: 2 * t + 2, :], xh[:, 2 * t: 2 * t + 2, :])
                        for t in (2 * half, 2 * half + 1):
                            mm(1, v, wt[:, KT + 2 * t: KT + 2 * t + 2, :], xh[:, 2 * t: 2 * t + 2, :])
                    for t in range(KT // 2):     # lh: w_lo x x_hi
                        mm(0, u, wt[:, 2 * KT + 2 * t: 2 * KT + 2 * t + 2, :], xh[:, 2 * t: 2 * t + 2, :])
                    for t in range(KT // 2):
                        mm(1, v, wt[:, 3 * KT + 2 * t: 3 * KT + 2 * t + 2, :], xh[:, 2 * t: 2 * t + 2, :])

                def hl_epilogue(i):
                    wt = w13ts[j, i]
                    u, v, ucnt, vcnt = psums.pop(i)
                    for t in range(KT // 2):     # hl: w_hi x x_lo
                        ucnt += 1
                        nc.tensor.matmul(u[:], wt[:, 2 * t: 2 * t + 2, :],
                                         xl[:, 2 * t: 2 * t + 2, :],
                                         start=False, stop=(ucnt == 12), perf_mode=DR)
                    for t in range(KT // 2):
                        vcnt += 1
                        nc.tensor.matmul(v[:], wt[:, KT + 2 * t: KT + 2 * t + 2, :],
                                         xl[:, 2 * t: 2 * t + 2, :],
                                         start=False, stop=(vcnt == 12), perf_mode=DR)
                    assert ucnt == 12 and vcnt == 12

                    su = sup.tile([128, nr], F32, tag="su")
                    nc.scalar.activation(out=su[:], in_=u[:],
                                         func=mybir.ActivationFunctionType.Silu,
                                         scale=1.0 / S1)
                    ht = htp.tile([128, nr], F32, tag="ht")
                    nc.vector.tensor_mul(ht[:], su[:], v[:])      # = 16*h_true
                    nc.vector.tensor_copy(out=hts[j][:, i, :], in_=ht[:])         # h_hi
                    nc.vector.tensor_sub(hts[j][:, IT + i, :], ht[:], hts[j][:, i, :])  # h_lo

                if j == 0:   # interleave i0/i1 so hi-part matmuls cover the
                    hh_lh(0)  # arrival gap of the lo-part DMAs at the head
                    hh_lh(1)
                    hl_epilogue(0)
                    hl_epilogue(1)
                    rest = range(2, IT)
                else:
                    rest = range(IT)
                for i in rest:
                    hh_lh(i)
                    hl_epilogue(i)

            w2ts = {}

            def load_w2(j):
                w2ts[j] = []
                for q in range(MT // 2):
                    wt = w2p.tile([128, 2, 2 * IT + 1, BLK], FP8, tag="w2")
                    nc.sync.dma_start(out=wt[:], in_=w2s[j][q])
                    w2ts[j].append(wt)

            def stage2(j):
                nr = CHUNKS[j]
                h = hts[j]
                pack = CHUNKS[j] < 4 * BLK
                oc = None
                for m in range(MT):
                    wt = w2ts[j][m // 2][:, m % 2]
                    o = op.tile([128, nr], F32, tag="o")
                    n_inst = 16
                    cnt = 0

                    def mm(st, mv):
                        nonlocal cnt
                        cnt += 1
                        nc.tensor.matmul(o[:], st, mv,
                                         start=(cnt == 1), stop=(cnt == n_inst),
                                         perf_mode=DR)

                    for t in range(5):   # hh pairs: blocks (0,1)..(8,9)
                        mm(wt[:, 2 * t: 2 * t + 2, :], h[:, 2 * t: 2 * t + 2, :])
                    # S1: w2_hi10 x (h_hi10 + h_lo10)
                    mm(wt[:, 10:11, :].broadcast_to((128, 2, BLK)),
                       h[:, 10: IT + 11: IT, :])
                    for t in range(5):   # lh pairs: w2_lo (0,1)..(8,9) x h_hi
                        mm(wt[:, IT + 2 * t: IT + 2 * t + 2, :], h[:, 2 * t: 2 * t + 2, :])
                    # S2: (w2_lo10 | w2_hi8-dup) x (h_hi10 | h_lo8)
                    mm(wt[:, 2 * IT - 1: 2 * IT + 1, :], h[:, 10: 2 * IT - 2: IT - 2, :])
                    for t in range(4):   # hl pairs: w2_hi (0,1)..(6,7) x h_lo
                        mm(wt[:, 2 * t: 2 * t + 2, :], h[:, IT + 2 * t: IT + 2 * t + 2, :])
                    assert cnt == n_inst

                    if not pack:
                        oc = ocp.tile([128, nr], FP16, tag="oc")
                        nc.scalar.mul(oc[:], o[:], OUT_DESCALE)
                        nc.sync.dma_start(out=outs[j][m], in_=oc[:])
                    else:
                        if m % 2 == 0:
                            oc = ocp.tile([128, 2 * nr], FP16, tag="oc2")
                        nc.scalar.mul(oc[:, (m % 2) * nr: (m % 2) * nr + nr], o[:], OUT_DESCALE)
                        if m % 2 == 1:
                            nc.sync.dma_start(out=outs[j][m // 2], in_=oc[:])

            for j in range(len(CHUNKS)):
                stage1(j)
            for j in (1, 0, 2):
                load_w2(j)
            for j in (1, 0, 2):
                stage2(j)

    nc.finalize()
    return nc


# ----------------------------------------------------------------------------
# Host-side data prep
# ----------------------------------------------------------------------------

def _hilo(a):
    hi = a.astype(E4)
    lo = (a - hi.astype(np.float32)).astype(E4)
    return hi, lo


def _fmt_w13(w1e, w3e, cols):
    """[H, I] x2 -> [IT, 128, 4*KT, 128] (w1hi | w3hi | w1lo | w3lo)."""
    out = np.empty((IT, 128, 4 * KT, BLK), dtype=E4)
    for w, base, s in ((w1e, 0, S1), (w3e, KT, S3)):
        half = np.ascontiguousarray(w[:, cols]) * s
        # [H, IHALF] -> [k, p, i, m] -> [i, p, k, m]
        r = half.reshape(KT, BLK, IT, BLK).transpose(2, 1, 0, 3)
        hi, lo = _hilo(r.astype(np.float32))
        out[:, :, base: base + KT, :] = hi
        out[:, :, 2 * KT + base: 2 * KT + base + KT, :] = lo
    return np.ascontiguousarray(out)


def _fmt_w2(w2e, rows):
    """[I, H] -> [MT, 128, 2*IT, 128] (hi blocks | lo blocks)."""
    half = np.ascontiguousarray(w2e[rows, :]) * S2
    r = half.reshape(IT, BLK, MT, BLK).transpose(2, 1, 0, 3)  # [m, p, b, q]
    hi, lo = _hilo(r.astype(np.float32))
    out = np.empty((MT, 128, 2 * IT + 1, BLK), dtype=E4)
    out[:, :, :IT, :] = hi
    out[:, :, IT: 2 * IT, :] = lo
    out[:, :, 2 * IT, :] = hi[:, :, 8, :]
    return np.ascontiguousarray(
        out.reshape(MT // 2, 2, 128, 2 * IT + 1, BLK).transpose(0, 2, 1, 3, 4))


def _fmt_x(xtoks):
    """[nr, H] -> hi and lo arrays [128, KT, nr]."""
    nr = xtoks.shape[0]
    r = np.ascontiguousarray(xtoks.T).reshape(KT, BLK, nr).transpose(1, 0, 2)  # [p,k,t]
    hi, lo = _hilo(r.astype(np.float32))
    return np.ascontiguousarray(hi), np.ascontiguousarray(lo)


_CACHE = {}


def _get_runner(gs_key, group_sizes):
    if gs_key in _CACHE:
        return _CACHE[gs_key]
    cells = _plan_cells(group_sizes)
    if cells is None:
        raise NotImplementedError(f"group_sizes {list(group_sizes)} not plannable")
    nc = _build_program()
    runner = _make_pjrt_runner(nc)
    st = {"nc": nc, "runner": runner, "cells": cells}
    _CACHE[gs_key] = st
    return st


def _make_pjrt_runner(nc):
    """Persistent jit'd SPMD executor (mirrors bass2jax.run_bass_via_pjrt)."""
    from jax.sharding import Mesh, PartitionSpec
    from jax.experimental.shard_map import shard_map
    from concourse.bass2jax import (
        _bass_exec_p, install_neuronx_cc_hook, partition_id_tensor,
    )

    install_neuronx_cc_hook()

    partition_name = nc.partition_id_tensor.name if nc.partition_id_tensor else None
    in_names, out_names, out_avals = [], [], []
    for alloc in nc.m.functions[0].allocations:
        if not isinstance(alloc, mybir.MemoryLocationSet):
            continue
        name = alloc.memorylocations[0].name
        if alloc.kind == "ExternalInput":
            if name != partition_name:
                in_names.append(name)
        elif alloc.kind == "ExternalOutput":
            out_names.append(name)
            out_avals.append(
                jax.core.ShapedArray(tuple(alloc.tensor_shape), mybir.dt.np(alloc.dtype))
            )
    n_params = len(in_names)
    n_outs = len(out_names)
    all_in_names = list(in_names) + list(out_names)
    if partition_name is not None:
        all_in_names.append(partition_name)
    donate = tuple(range(n_params, n_params + n_outs))

    def _body(*args):
        operands = list(args)
        if partition_name is not None:
            operands.append(partition_id_tensor())
        outs = _bass_exec_p.bind(
            *operands,
            out_avals=tuple(out_avals),
            in_names=tuple(all_in_names),
            out_names=tuple(out_names),
            lowering_input_output_aliases=(),
            sim_require_finite=True,
            sim_require_nnan=True,
            nc=nc,
        )
        return tuple(outs)

    devices = jax.devices()[:NCORES]
    mesh = Mesh(np.asarray(devices), ("core",))
    in_specs = (PartitionSpec("core"),) * (n_params + n_outs)
    out_specs = (PartitionSpec("core"),) * n_outs
    jitted = jax.jit(
        shard_map(_body, mesh=mesh, in_specs=in_specs, out_specs=out_specs,
                  check_rep=False),
        donate_argnums=donate, keep_unused=True,
    )

    def run(in_maps):
        per_core = [[np.asarray(m[n]) for n in in_names] for m in in_maps]
        concat_in = [
            np.concatenate([per_core[c][i] for c in range(NCORES)], axis=0)
            for i in range(n_params)
        ]
        zeros = [
            np.zeros((NCORES * a.shape[0], *a.shape[1:]), a.dtype) for a in out_avals
        ]
        out_arrs = jitted(*concat_in, *zeros)
        return [
            {
                name: np.asarray(out_arrs[i]).reshape(NCORES, *out_avals[i].shape)[c]
                for i, name in enumerate(out_names)
            }
            for c in range(NCORES)
        ]

    return run


def _cell_cols(side):
    return slice(0, IHALF) if side == "L" else slice(IHALF, II)


def _prep_in_maps(st, hidden_states, w1, w2, w3):
    cells = st["cells"]
    hs = np.asarray(hidden_states, dtype=np.float32)
    w1 = np.asarray(w1, dtype=np.float32)
    w2 = np.asarray(w2, dtype=np.float32)
    w3 = np.asarray(w3, dtype=np.float32)

    in_maps = [dict() for _ in range(NCORES)]
    size_of_chunk = {512: 4, 384: 3, 128: 1}
    for j, nr in enumerate(CHUNKS):
        blk = size_of_chunk[nr]
        for c in range(NCORES):
            e, t0, side = cells[blk][c]
            cols = _cell_cols(side)
            xh, xl = _fmt_x(hs[t0 * BLK: t0 * BLK + nr])
            in_maps[c][f"xh_{j}"] = xh
            in_maps[c][f"xl_{j}"] = xl
            in_maps[c][f"w13_{j}"] = _fmt_w13(w1[e], w3[e], cols)
            in_maps[c][f"w2_{j}"] = _fmt_w2(w2[e], cols)
    return in_maps


def _assemble(st, results, out_dtype):
    cells = st["cells"]
    out = np.zeros((4096, H), dtype=np.float32)
    size_of_chunk = {512: 4, 384: 3, 128: 1}
    for j, nr in enumerate(CHUNKS):
        blk = size_of_chunk[nr]
        for c in range(NCORES):
            e, t0, side = cells[blk][c]
            part = results[c][f"out_{j}"].astype(np.float32)
            if nr < 4 * BLK:   # packed m-tile pairs: [MT//2, 128, 2*nr]
                part = part.reshape(MT // 2, 128, 2, nr).transpose(0, 2, 1, 3)
            part = part.reshape(H, nr).T                      # [nr, H]
            out[t0 * BLK: t0 * BLK + nr] += part
    return out.astype(out_dtype)


def kernel(hidden_states, group_sizes, w1, w2, w3):
    gs = np.asarray(group_sizes)
    st = _get_runner(gs.tobytes(), gs)
    in_maps = _prep_in_maps(st, hidden_states, w1, w2, w3)
    results = st["runner"](in_maps)
    return _assemble(st, results, np.asarray(hidden_states).dtype)
